# revision 10
# baseline (speedup 1.0000x reference)
"""Trainium2 Bass kernel for AdaptedEnzymeModel (per-node MLP -> segment mean
pool -> graph MLP), SPMD over 8 NeuronCores.  v2.1: blockdiag layers, padded
16-node block pooling, software-pipelined supers.

Design
------
* BN (eval) affines folded into adjacent Linears on host; device runs bf16
  Linear+ReLU chains with fp32 PSUM accumulate.
* Nodes sharded at graph boundaries: core c owns graphs [512c, 512c+512),
  split into 4 groups of 128 graphs.  Each graph is padded to a multiple of
  16 nodes (pad nodes x=0); each group padded to G2 (multiple of 2048).
* Per 2048-node "super": L1 is ONE 64x128x512 matmul (4 channels x 32 feats
  packed in partitions; selector stationary picks the 4 x-rows).  L2-L4 run
  as full-array matmuls with block-diagonal stationaries diag(W,W) on the
  dual-channel [128,512] layout.  L5/L6 are weight-stationary, 4 matmuls each
  into one 4-bank [128,2048] PSUM tile, evacuated by a single ACT op
  (relu + per-partition bias).
* Pooling: one DVE tensor_reduce per super sums each 16-col block of z6
  ([128,128,16] -> [128,128] bf16) into the group block-sum buffer; that
  128-col chunk is immediately 128x128-transposed by the DMA xbar.  Pad-node
  garbage (x=0 -> constant c6 vector) is removed exactly by a rank-1
  c6 (x) (-padcount) correction matmul.  End phase: per group 16 accumulating
  matmuls against the host-built block->graph interval matrix -> fp32 means
  -> graph MLP -> [7, 512] out.
* Emission is software-pipelined: iteration k issues L1-L4 of super k,
  L5/L6 of super k-1 and the reduce+transpose of super k-2, so ACT/DVE
  queues always hold ready work.
"""

import numpy as np
import ml_dtypes
from contextlib import ExitStack

import concourse.bass as bass
import concourse.tile as tile
from concourse import bacc, mybir
from concourse.bass_utils import run_bass_kernel_spmd

NCORES = 8
GROUPS = 4
BINS = 128
GRAN = 16
SUP = 2048
NCLS = 7
EPS = 1e-5
F32 = mybir.dt.float32
BF16 = mybir.dt.bfloat16
FP16 = mybir.dt.float16
NPBF = ml_dtypes.bfloat16
RELU = mybir.ActivationFunctionType.Relu
ALU = mybir.AluOpType
AXX = mybir.AxisListType.X

LAST_RESULT = None
_NC_CACHE = {}


def _ensure_ntff_hook():
    """bass_utils' trace path needs antenv.axon_hooks, which this image's
    antenv package lacks.  Register a shim backed by trn_agent_boot's ctypes
    NTFF driver so BASS_TRACE=1 yields exec_time_ns.  Degrades silently."""
    import sys
    import types
    try:
        import antenv
        if "antenv.axon_hooks" in sys.modules:
            return
        mod = types.ModuleType("antenv.axon_hooks")
        mod._hook = None
        mod.set_axon_ntff_profile_hook = lambda h: setattr(mod, "_hook", h)
        mod.get_axon_ntff_profile_hook = lambda: mod._hook
        sys.modules["antenv.axon_hooks"] = mod
        antenv.axon_hooks = mod
        from trn_agent_boot.trn_boot import _ntff_profile_via_ctypes
        mod._hook = _ntff_profile_via_ctypes("/opt/axon/libaxon_pjrt.so")
    except Exception:
        pass


_ensure_ntff_hook()


# ---------------------------------------------------------------- host math --
def _fold(p):
    def aff(bn):
        g, b, m, v = bn[0], bn[1], bn[2], bn[3]
        s = g / np.sqrt(v + EPS)
        return s.astype(np.float32), (b - m * s).astype(np.float32)

    s1, t1 = aff(p["ne_bn1"]); s2, t2 = aff(p["ne_bn2"])
    sc1, tc1 = aff(p["cbn1"]); sc2, tc2 = aff(p["cbn2"])
    sf1, tf1 = aff(p["fbn1"]); sf2, tf2 = aff(p["fbn2"])
    f = {}
    f["W1"] = p["ne_w1"]; f["B1"] = p["ne_b1"]
    f["W2"] = s1[:, None] * p["ne_w2"]; f["B2"] = t1 @ p["ne_w2"] + p["ne_b2"]
    f["W3"] = s2[:, None] * p["c1a_w"]; f["B3"] = t2 @ p["c1a_w"] + p["c1a_b"]
    f["W4"] = p["c1b_w"];               f["B4"] = p["c1b_b"]
    f["W5"] = sc1[:, None] * p["c2a_w"]; f["B5"] = tc1 @ p["c2a_w"] + p["c2a_b"]
    f["W6"] = p["c2b_w"];               f["B6"] = p["c2b_b"]
    f["F1"] = sc2[:, None] * p["f1_w"]; f["F1B"] = tc2 @ p["f1_w"] + p["f1_b"]
    f["F2"] = sf1[:, None] * p["f2_w"]; f["F2B"] = tf1 @ p["f2_w"] + p["f2_b"]
    f["F3"] = sf2[:, None] * p["f3_w"]; f["F3B"] = tf2 @ p["f3_w"] + p["f3_b"]
    return {k: np.asarray(v, np.float32) for k, v in f.items()}


def _c6(f):
    """Feature vector a pad node (x=0) produces at z6, replicating device
    rounding (bf16 weights/activations, fp32 accumulate)."""
    relu = lambda a: np.maximum(a, 0.0)
    z = relu(f["B1"]).astype(NPBF)
    for w, b in (("W2", "B2"), ("W3", "B3"), ("W4", "B4"), ("W5", "B5"),
                 ("W6", "B6")):
        z = z.astype(np.float32) @ f[w].astype(NPBF).astype(np.float32)
        z = relu(z + f[b]).astype(NPBF)
    return z.astype(np.float32)


# bf16 const block layout
def _layout_bf():
    off, c = {}, 0
    for name, ncols in [("BD2A", 128), ("BD2B", 128), ("BD3", 128),
                        ("BD4", 128), ("W5R", 128), ("W6", 128),
                        ("F1", 64), ("F2", 32), ("F3", NCLS),
                        ("C6", 128), ("NEG", GROUPS * BINS)]:
        off[name] = c
        c += ncols
    return off, c


def _layout_fp():
    off, c = {}, 0
    for name, ncols in [("B1S", 1), ("B2S", 1), ("B3S", 1), ("B4S", 1),
                        ("B5S", 1), ("B6S", 1), ("F1B", 1), ("F2B", 1),
                        ("F3B", 1), ("INV", GROUPS * BINS)]:
        off[name] = c
        c += ncols
    return off, c


_OFFB, _CWB = _layout_bf()
_OFFF, _CWF = _layout_fp()


def _pack_consts(f, c6, negpad, inv):
    """negpad [512] f32, inv [512] f32 per core."""
    wb = np.zeros((128, _CWB), NPBF)

    def putb(name, arr):
        wb[:arr.shape[0], _OFFB[name]:_OFFB[name] + arr.shape[1]] = \
            arr.astype(NPBF)

    bd2a = np.zeros((128, 128), np.float32)
    bd2a[0:32, 0:64] = f["W2"]
    bd2a[32:64, 64:128] = f["W2"]
    bd2b = np.zeros((128, 128), np.float32)
    bd2b[64:96, 0:64] = f["W2"]
    bd2b[96:128, 64:128] = f["W2"]
    putb("BD2A", bd2a)
    putb("BD2B", bd2b)
    for nm, w in (("BD3", "W3"), ("BD4", "W4")):
        bd = np.zeros((128, 128), np.float32)
        bd[0:64, 0:64] = f[w]
        bd[64:128, 64:128] = f[w]
        putb(nm, bd)
    putb("W5R", np.tile(f["W5"], (2, 1)))
    putb("W6", f["W6"])
    putb("F1", f["F1"])
    putb("F2", f["F2"])
    putb("F3", f["F3"])
    wb[0, _OFFB["C6"]:_OFFB["C6"] + 128] = c6.astype(NPBF)
    wb[0, _OFFB["NEG"]:_OFFB["NEG"] + GROUPS * BINS] = negpad.astype(NPBF)

    wf = np.zeros((128, _CWF), np.float32)
    wf[:, _OFFF["B1S"]] = np.tile(f["B1"], 4)
    wf[:, _OFFF["B2S"]] = np.tile(f["B2"], 2)
    wf[:, _OFFF["B3S"]] = np.tile(f["B3"], 2)
    wf[:, _OFFF["B4S"]] = np.tile(f["B4"], 2)
    wf[:, _OFFF["B5S"]] = f["B5"]
    wf[:, _OFFF["B6S"]] = f["B6"]
    wf[:64, _OFFF["F1B"]] = f["F1B"]
    wf[:32, _OFFF["F2B"]] = f["F2B"]
    wf[:NCLS, _OFFF["F3B"]] = f["F3B"]
    wf[:, _OFFF["INV"]:_OFFF["INV"] + GROUPS * BINS] = inv[None, :]
    return wb, wf


def _pack_sel(f, nsup):
    """L1 selector: col block s = [64, 128]; out partition 32c+j gets
    W1[0, j] from x-row (4s+c)."""
    sel = np.zeros((64, nsup * 128), NPBF)
    w1 = f["W1"][0].astype(NPBF)
    for s in range(nsup):
        for c in range(4):
            r = (4 * s + c) % 64
            sel[r, s * 128 + 32 * c: s * 128 + 32 * c + 32] = w1
    return sel


# ------------------------------------------------------------- device build --
def _build(G2):
    NSUP = G2 // SUP
    NBLK = G2 // GRAN
    BPS = SUP // GRAN            # blocks per super (=128)
    assert BPS == 128 and G2 <= 64 * 512
    NCHUNK = NBLK // 128         # == NSUP

    nc = bacc.Bacc(None, target_bir_lowering=False)
    xs_d = nc.declare_dram_parameter("xs", [GROUPS, 64, 512], BF16,
                                     isOutput=False)
    a_d = nc.declare_dram_parameter("amat", [GROUPS, 128, NBLK], BF16,
                                    isOutput=False)
    sel_d = nc.declare_dram_parameter("selc", [64, NSUP * 128], BF16,
                                      isOutput=False)
    wb_d = nc.declare_dram_parameter("wbf", [128, _CWB], BF16, isOutput=False)
    wf_d = nc.declare_dram_parameter("wfp", [128, _CWF], F32, isOutput=False)
    out_d = nc.declare_dram_parameter("out", [NCLS, GROUPS * BINS], F32,
                                      isOutput=True)

    with ExitStack() as ctx:
        tc = ctx.enter_context(tile.TileContext(nc))
        cpool = ctx.enter_context(tc.tile_pool(name="const", bufs=1))
        xpool = ctx.enter_context(tc.tile_pool(name="xg", bufs=2))
        zpool = ctx.enter_context(tc.tile_pool(name="zq", bufs=2))
        gpool = ctx.enter_context(tc.tile_pool(name="gacc", bufs=1))

        wbsb = cpool.tile([128, _CWB], BF16)
        nc.sync.dma_start(wbsb[:], wb_d[:])
        wfsb = cpool.tile([128, _CWF], F32)
        nc.sync.dma_start(wfsb[:], wf_d[:])
        selsb = cpool.tile([64, NSUP * 128], BF16)
        nc.sync.dma_start(selsb[:], sel_d[:])

        def WB(name, k, m):
            o = _OFFB[name]
            return wbsb[0:k, o:o + m]

        def WF(name, k, m=1):
            o = _OFFF[name]
            return wfsb[0:k, o:o + m]

        bd2a, bd2b = WB("BD2A", 128, 128), WB("BD2B", 128, 128)
        bd3, bd4 = WB("BD3", 128, 128), WB("BD4", 128, 128)
        w5r, w6 = WB("W5R", 128, 128), WB("W6", 128, 128)
        f1, f2, f3 = WB("F1", 128, 64), WB("F2", 64, 32), WB("F3", 32, NCLS)
        c6row = WB("C6", 1, 128)
        negrow = WB("NEG", 1, GROUPS * BINS)
        b1s, b2s, b3s = WF("B1S", 128), WF("B2S", 128), WF("B3S", 128)
        b4s, b5s, b6s = WF("B4S", 128), WF("B5S", 128), WF("B6S", 128)
        f1b, f2b, f3b = WF("F1B", 64), WF("F2B", 32), WF("F3B", NCLS)
        invsb = WF("INV", 128, GROUPS * BINS)

        bs_t, bt_t, a_t = [], [], []
        for g in range(GROUPS):
            bs_t.append(gpool.tile([128, NBLK], BF16, name=f"bs{g}"))
            bt_t.append(gpool.tile([128, NBLK], BF16, name=f"bt{g}"))
            a_t.append(gpool.tile([128, NBLK], BF16, name=f"amat{g}"))
        gsb = gpool.tile([128, GROUPS * BINS], BF16, name="gsb")

        for g in range(GROUPS):
            nc.sync.dma_start(a_t[g][:], a_d[g])

        supers = [(g, s) for g in range(GROUPS) for s in range(NSUP)]
        K = len(supers)
        st = {}          # k -> dict of live tiles
        xgs = {}

        def load_x(g):
            xg = xpool.tile([64, 512], BF16, tag="xg", name=f"xg{g}")
            nc.sync.dma_start(xg[:], xs_d[g])
            xgs[g] = xg

        def stage_A(k):
            g, s = supers[k]
            d = st.setdefault(k, {})
            p1 = psS.tile([128, 1024], F32, tag="ps", name=f"p1_{k}")
            nc.tensor.matmul(p1[:, 0:512], selsb[:, s * 128:(s + 1) * 128],
                             xgs[g][:], start=True, stop=True)
            z1q = zpool.tile([128, 512], BF16, tag="z1", name=f"z1_{k}")
            if k % 2 == 0:
                nc.scalar.activation(z1q[:], p1[:, 0:512], RELU, bias=b1s)
            else:
                nc.vector.tensor_scalar(z1q[:], p1[:, 0:512], b1s, 0.0,
                                        ALU.add, ALU.max)
            d["z1"] = z1q

        def stage_L2(k):
            d = st[k]
            p2 = psS.tile([128, 1024], F32, tag="ps", name=f"p2_{k}")
            nc.tensor.matmul(p2[:, 0:512], bd2a, d["z1"][:],
                             start=True, stop=True)
            nc.tensor.matmul(p2[:, 512:1024], bd2b, d["z1"][:],
                             start=True, stop=True)
            z2q = zpool.tile([128, 1024], BF16, tag="z2", name=f"z2_{k}",
                             bufs=3)
            nc.vector.tensor_scalar(z2q[:], p2[:], b2s, 0.0, ALU.add, ALU.max)
            d["z2"] = z2q

        def stage_L3(k):
            d = st[k]
            p3 = psS.tile([128, 1024], F32, tag="ps", name=f"p3_{k}")
            nc.tensor.matmul(p3[:, 0:512], bd3, d["z2"][:, 0:512],
                             start=True, stop=True)
            nc.tensor.matmul(p3[:, 512:1024], bd3, d["z2"][:, 512:1024],
                             start=True, stop=True)
            z3q = zpool.tile([128, 1024], BF16, tag="z3", name=f"z3_{k}",
                             bufs=3)
            nc.vector.tensor_scalar(z3q[:], p3[:], b3s, 0.0, ALU.add, ALU.max)
            d["z3"] = z3q

        def stage_L4(k):
            d = st[k]
            p4 = psS.tile([128, 1024], F32, tag="ps", name=f"p4_{k}")
            nc.tensor.matmul(p4[:, 0:512], bd4, d["z3"][:, 0:512],
                             start=True, stop=True)
            nc.tensor.matmul(p4[:, 512:1024], bd4, d["z3"][:, 512:1024],
                             start=True, stop=True)
            z4q = zpool.tile([128, 1024], BF16, tag="z4", name=f"z4_{k}",
                             bufs=3)
            nc.vector.tensor_scalar(z4q[:], p4[:], b4s, 0.0, ALU.add, ALU.max)
            d["z4"] = z4q

        def stage_L5(k):
            d = st[k]
            p5 = psB.tile([128, 2048], F32, tag="big", name=f"p5_{k}")
            z4q = d["z4"]
            for ch in range(4):
                lo, hi = (0, 64) if ch % 2 == 0 else (64, 128)
                col = (ch // 2) * 512
                nc.tensor.matmul(p5[:, ch * 512:(ch + 1) * 512],
                                 w5r[lo:hi, :], z4q[lo:hi, col:col + 512],
                                 start=True, stop=True)
            z5q = zpool.tile([128, 2048], BF16, tag="z5", name=f"z5_{k}",
                             bufs=3)
            nc.scalar.activation(z5q[:], p5[:], RELU, bias=b5s)
            d["z5"] = z5q

        def stage_L6(k):
            d = st[k]
            p6 = psB.tile([128, 2048], F32, tag="big", name=f"p6_{k}")
            z5q = d["z5"]
            for ch in range(4):
                nc.tensor.matmul(p6[:, ch * 512:(ch + 1) * 512],
                                 w6, z5q[:, ch * 512:(ch + 1) * 512],
                                 start=True, stop=True)
            z6q = zpool.tile([128, 2048], BF16, tag="z6", name=f"z6_{k}",
                             bufs=3)
            nc.scalar.activation(z6q[:], p6[:], RELU, bias=b6s)
            d["z6"] = z6q

        def stage_R(k):
            """Fold-tree block sums: gpsimd does the two big folds, DVE the
            two small ones.  Blocks are strided (node m of block j sits at
            column j + 128*m of the super), so halving folds preserve them."""
            g, s = supers[k]
            z6q = st[k]["z6"]
            t1 = zpool.tile([128, 1024], FP16, tag="t1", name=f"t1_{k}")
            nc.gpsimd.tensor_tensor(t1[:], z6q[:, 0:1024], z6q[:, 1024:2048],
                                    ALU.add)
            t2 = zpool.tile([128, 512], FP16, tag="t2", name=f"t2_{k}")
            nc.gpsimd.tensor_tensor(t2[:], t1[:, 0:512], t1[:, 512:1024],
                                    ALU.add)
            t3 = zpool.tile([128, 256], FP16, tag="t3", name=f"t3_{k}")
            nc.gpsimd.tensor_tensor(t3[:], t2[:, 0:256], t2[:, 256:512],
                                    ALU.add)
            with nc.allow_low_precision("bf16 block sums; pooled means "
                                        "tolerate 0.4% noise"):
                nc.vector.tensor_tensor(bs_t[g][:, s * 128:(s + 1) * 128],
                                        t3[:, 0:128], t3[:, 128:256], ALU.add)
            nc.sync.dma_start_transpose(
                bt_t[g][:, s * 128:(s + 1) * 128],
                bs_t[g][:, s * 128:(s + 1) * 128])
            del st[k]

        def agg_group(g, pool, tag):
            sgt = pool.tile([128, 2048], F32, tag=tag, name=f"sg{g}") \
                if tag == "big" else pool.tile([128, BINS], F32, tag=tag,
                                               name=f"sg{g}")
            sg = sgt[:, 0:BINS]
            for c in range(NCHUNK):
                nc.tensor.matmul(sg, bt_t[g][:, c * 128:(c + 1) * 128],
                                 a_t[g][:, c * 128:(c + 1) * 128],
                                 start=(c == 0), stop=False,
                                 skip_group_check=True)
            nc.tensor.matmul(sg, c6row, negrow[:, g * BINS:(g + 1) * BINS],
                             start=False, stop=True, skip_group_check=True)
            nc.vector.tensor_tensor(
                gsb[:, g * BINS:(g + 1) * BINS], sg,
                invsb[:, g * BINS:(g + 1) * BINS], ALU.mult)

        with tc.tile_pool(name="psS", bufs=2, space="PSUM") as psS, \
             tc.tile_pool(name="psB", bufs=1, space="PSUM") as psB:
            load_x(0)
            for k in range(K + 6):
                if k < K:
                    if k + 4 < K and supers[k + 4][1] == 0:
                        load_x(supers[k + 4][0])
                    stage_A(k)
                if 0 <= k - 1 < K:
                    stage_L3(k - 1)
                if 0 <= k - 2 < K:
                    stage_L4(k - 2)
                if 0 <= k - 3 < K:
                    stage_L5(k - 3)
                if 0 <= k - 4 < K:
                    stage_L6(k - 4)
                if 0 <= k - 5 < K:
                    stage_R(k - 5)
                if k < K:
                    stage_L2(k)
                # group aggregation rides the big-psum slot once its last
                # block-sum chunk has been DMA-transposed (k_last + 5 < k)
                if k >= NSUP + 7 and (k - 7) % NSUP == 0:
                    g_done = (k - 7) // NSUP - 1
                    if g_done < GROUPS - 1:
                        agg_group(g_done, psB, "big")

        # ---------------- end phase: last group + graph MLP ----------------
        with tc.tile_pool(name="psA", bufs=2, space="PSUM") as psA:
            agg_group(GROUPS - 1, psA, "agg")

            pf1 = psA.tile([64, 512], F32, tag="agg", name="pf1")
            nc.tensor.matmul(pf1[:], f1, gsb[:], start=True, stop=True)
            a1 = zpool.tile([64, 512], BF16, tag="a1")
            nc.scalar.activation(a1[:], pf1[:], RELU, bias=f1b)
            pf2 = psA.tile([32, 512], F32, tag="agg", name="pf2")
            nc.tensor.matmul(pf2[:], f2, a1[:], start=True, stop=True)
            a2 = zpool.tile([32, 512], BF16, tag="a2")
            nc.scalar.activation(a2[:], pf2[:], RELU, bias=f2b)
            pf3 = psA.tile([NCLS, 512], F32, tag="agg", name="pf3")
            nc.tensor.matmul(pf3[:], f3, a2[:], start=True, stop=True)
            osb = zpool.tile([NCLS, 512], F32, tag="osb")
            nc.vector.tensor_scalar(osb[:], pf3[:], f3b, None, ALU.add)
            nc.sync.dma_start(out_d[:], osb[:])

    nc.compile()
    return nc


# -------------------------------------------------------------------- entry --
def kernel(**inputs):
    global LAST_RESULT
    x = np.asarray(inputs["x"], np.float32)
    batch = np.asarray(inputs["batch"], np.int64)
    B = int(np.asarray(inputs["num_graphs"]))
    assert B == NCORES * GROUPS * BINS, f"unexpected num_graphs {B}"
    T = x.shape[0]

    params = {k: np.asarray(v, np.float32) for k, v in inputs.items()
              if k not in ("x", "batch", "num_graphs")}
    f = _fold(params)
    c6 = _c6(f)

    counts = np.bincount(batch, minlength=B).astype(np.int64)
    nblk = -(-counts // GRAN)
    pad = (nblk * GRAN - counts).astype(np.float32)
    NCG = NCORES * GROUPS
    nblk_cg = nblk.reshape(NCG, BINS)
    blkstart = np.zeros((NCG, BINS), np.int64)
    blkstart[:, 1:] = np.cumsum(nblk_cg, axis=1)[:, :-1]
    P_cg = nblk_cg.sum(axis=1) * GRAN
    G2 = int(-(-int(P_cg.max()) // SUP) * SUP)
    NBLK = G2 // GRAN

    # padded positions
    bounds = np.zeros(B + 1, np.int64)
    bounds[1:] = np.cumsum(counts)
    within = np.arange(T, dtype=np.int64) - bounds[batch]
    cg_of = batch // BINS
    ppos = blkstart[cg_of, batch % BINS] * GRAN + within
    # strided in-super layout: node m of block j -> column j + 128*m, so the
    # device fold-tree (halving adds) preserves block identity
    q = ppos % SUP
    dpos = (ppos // SUP) * SUP + (q % GRAN) * (SUP // GRAN) + q // GRAN
    xp = np.zeros((NCG, 64 * 512), np.float32)
    xp[cg_of, dpos] = x
    xs = xp.reshape(NCORES, GROUPS, 64, 512).astype(NPBF)

    # block -> bin interval matrix, chunk-transposed device layout
    amat = np.zeros((NCG, NBLK, BINS), NPBF)
    for cg in range(NCG):
        owner = np.repeat(np.arange(BINS), nblk_cg[cg])
        amat[cg, np.arange(owner.size), owner] = NPBF(1.0)
    amat = amat.reshape(NCORES, GROUPS, NBLK // 128, 128, BINS)
    amat = np.ascontiguousarray(amat.transpose(0, 1, 3, 2, 4)).reshape(
        NCORES, GROUPS, 128, NBLK)

    negpad = (-pad).reshape(NCORES, GROUPS * BINS)
    inv = (1.0 / np.maximum(counts, 1)).astype(np.float32).reshape(
        NCORES, GROUPS * BINS)

    sel = _pack_sel(f, G2 // SUP)

    if G2 not in _NC_CACHE:
        _NC_CACHE[G2] = _build(G2)
    nc = _NC_CACHE[G2]

    in_maps = []
    for c in range(NCORES):
        wb, wf = _pack_consts(f, c6, negpad[c], inv[c])
        in_maps.append({"xs": xs[c], "amat": amat[c], "selc": sel,
                       "wbf": wb, "wfp": wf})
    res = run_bass_kernel_spmd(nc, in_maps, core_ids=list(range(NCORES)))
    LAST_RESULT = res
    outs = np.stack([res.results[i]["out"] for i in range(NCORES)])
    return np.ascontiguousarray(
        outs.transpose(0, 2, 1).reshape(B, NCLS)).astype(np.float32)


# revision 12
# speedup vs baseline: 1.0791x; 1.0791x over previous
"""Trainium2 Bass kernel for AdaptedEnzymeModel (per-node MLP -> segment mean
pool -> graph MLP), SPMD over 8 NeuronCores.  v2.1: blockdiag layers, padded
16-node block pooling, software-pipelined supers.

Design
------
* BN (eval) affines folded into adjacent Linears on host; device runs bf16
  Linear+ReLU chains with fp32 PSUM accumulate.
* Nodes sharded at graph boundaries: core c owns graphs [512c, 512c+512),
  split into 4 groups of 128 graphs.  Each graph is padded to a multiple of
  16 nodes (pad nodes x=0); each group padded to G2 (multiple of 2048).
* Per 2048-node "super": L1 is ONE 64x128x512 matmul (4 channels x 32 feats
  packed in partitions; selector stationary picks the 4 x-rows).  L2-L4 run
  as full-array matmuls with block-diagonal stationaries diag(W,W) on the
  dual-channel [128,512] layout.  L5/L6 are weight-stationary, 4 matmuls each
  into one 4-bank [128,2048] PSUM tile, evacuated by a single ACT op
  (relu + per-partition bias).
* Pooling: one DVE tensor_reduce per super sums each 16-col block of z6
  ([128,128,16] -> [128,128] bf16) into the group block-sum buffer; that
  128-col chunk is immediately 128x128-transposed by the DMA xbar.  Pad-node
  garbage (x=0 -> constant c6 vector) is removed exactly by a rank-1
  c6 (x) (-padcount) correction matmul.  End phase: per group 16 accumulating
  matmuls against the host-built block->graph interval matrix -> fp32 means
  -> graph MLP -> [7, 512] out.
* Emission is software-pipelined: iteration k issues L1-L4 of super k,
  L5/L6 of super k-1 and the reduce+transpose of super k-2, so ACT/DVE
  queues always hold ready work.
"""

import numpy as np
import ml_dtypes
from contextlib import ExitStack

import concourse.bass as bass
import concourse.tile as tile
from concourse import bacc, mybir
from concourse.bass_utils import run_bass_kernel_spmd

NCORES = 8
GROUPS = 4
BINS = 128
GRAN = 16
SUP = 2048
NCLS = 7
EPS = 1e-5
F32 = mybir.dt.float32
BF16 = mybir.dt.bfloat16
FP16 = mybir.dt.float16
NPBF = ml_dtypes.bfloat16
RELU = mybir.ActivationFunctionType.Relu
ALU = mybir.AluOpType
AXX = mybir.AxisListType.X

LAST_RESULT = None
_NC_CACHE = {}


def _ensure_ntff_hook():
    """bass_utils' trace path needs antenv.axon_hooks, which this image's
    antenv package lacks.  Register a shim backed by trn_agent_boot's ctypes
    NTFF driver so BASS_TRACE=1 yields exec_time_ns.  Degrades silently."""
    import sys
    import types
    try:
        import antenv
        if "antenv.axon_hooks" in sys.modules:
            return
        mod = types.ModuleType("antenv.axon_hooks")
        mod._hook = None
        mod.set_axon_ntff_profile_hook = lambda h: setattr(mod, "_hook", h)
        mod.get_axon_ntff_profile_hook = lambda: mod._hook
        sys.modules["antenv.axon_hooks"] = mod
        antenv.axon_hooks = mod
        from trn_agent_boot.trn_boot import _ntff_profile_via_ctypes
        mod._hook = _ntff_profile_via_ctypes("/opt/axon/libaxon_pjrt.so")
    except Exception:
        pass


_ensure_ntff_hook()


# ---------------------------------------------------------------- host math --
def _fold(p):
    def aff(bn):
        g, b, m, v = bn[0], bn[1], bn[2], bn[3]
        s = g / np.sqrt(v + EPS)
        return s.astype(np.float32), (b - m * s).astype(np.float32)

    s1, t1 = aff(p["ne_bn1"]); s2, t2 = aff(p["ne_bn2"])
    sc1, tc1 = aff(p["cbn1"]); sc2, tc2 = aff(p["cbn2"])
    sf1, tf1 = aff(p["fbn1"]); sf2, tf2 = aff(p["fbn2"])
    f = {}
    f["W1"] = p["ne_w1"]; f["B1"] = p["ne_b1"]
    f["W2"] = s1[:, None] * p["ne_w2"]; f["B2"] = t1 @ p["ne_w2"] + p["ne_b2"]
    f["W3"] = s2[:, None] * p["c1a_w"]; f["B3"] = t2 @ p["c1a_w"] + p["c1a_b"]
    f["W4"] = p["c1b_w"];               f["B4"] = p["c1b_b"]
    f["W5"] = sc1[:, None] * p["c2a_w"]; f["B5"] = tc1 @ p["c2a_w"] + p["c2a_b"]
    f["W6"] = p["c2b_w"];               f["B6"] = p["c2b_b"]
    f["F1"] = sc2[:, None] * p["f1_w"]; f["F1B"] = tc2 @ p["f1_w"] + p["f1_b"]
    f["F2"] = sf1[:, None] * p["f2_w"]; f["F2B"] = tf1 @ p["f2_w"] + p["f2_b"]
    f["F3"] = sf2[:, None] * p["f3_w"]; f["F3B"] = tf2 @ p["f3_w"] + p["f3_b"]
    return {k: np.asarray(v, np.float32) for k, v in f.items()}


def _c6(f):
    """Feature vector a pad node (x=0) produces at z6, replicating device
    rounding (bf16 weights/activations, fp32 accumulate)."""
    relu = lambda a: np.maximum(a, 0.0)
    z = relu(f["B1"]).astype(NPBF)
    for w, b in (("W2", "B2"), ("W3", "B3"), ("W4", "B4"), ("W5", "B5"),
                 ("W6", "B6")):
        z = z.astype(np.float32) @ f[w].astype(NPBF).astype(np.float32)
        z = relu(z + f[b]).astype(NPBF)
    return z.astype(np.float32)


# bf16 const block layout
def _layout_bf():
    off, c = {}, 0
    for name, ncols in [("BD2A", 128), ("BD2B", 128), ("BD3", 128),
                        ("BD4", 128), ("W5R", 128), ("W6", 128),
                        ("F1", 64), ("F2", 32), ("F3", NCLS),
                        ("C6", 128), ("NEG", GROUPS * BINS)]:
        off[name] = c
        c += ncols
    return off, c


def _layout_fp():
    off, c = {}, 0
    for name, ncols in [("B1S", 1), ("B2S", 1), ("B3S", 1), ("B4S", 1),
                        ("B5S", 1), ("B6S", 1), ("F1B", 1), ("F2B", 1),
                        ("F3B", 1), ("INV", GROUPS * BINS)]:
        off[name] = c
        c += ncols
    return off, c


_OFFB, _CWB = _layout_bf()
_OFFF, _CWF = _layout_fp()


def _pack_consts(f, c6, negpad, inv):
    """negpad [512] f32, inv [512] f32 per core."""
    wb = np.zeros((128, _CWB), NPBF)

    def putb(name, arr):
        wb[:arr.shape[0], _OFFB[name]:_OFFB[name] + arr.shape[1]] = \
            arr.astype(NPBF)

    bd2a = np.zeros((128, 128), np.float32)
    bd2a[0:32, 0:64] = f["W2"]
    bd2a[32:64, 64:128] = f["W2"]
    bd2b = np.zeros((128, 128), np.float32)
    bd2b[64:96, 0:64] = f["W2"]
    bd2b[96:128, 64:128] = f["W2"]
    putb("BD2A", bd2a)
    putb("BD2B", bd2b)
    for nm, w in (("BD3", "W3"), ("BD4", "W4")):
        bd = np.zeros((128, 128), np.float32)
        bd[0:64, 0:64] = f[w]
        bd[64:128, 64:128] = f[w]
        putb(nm, bd)
    putb("W5R", np.tile(f["W5"], (2, 1)))
    putb("W6", f["W6"])
    putb("F1", f["F1"])
    putb("F2", f["F2"])
    putb("F3", f["F3"])
    wb[0, _OFFB["C6"]:_OFFB["C6"] + 128] = c6.astype(NPBF)
    wb[0, _OFFB["NEG"]:_OFFB["NEG"] + GROUPS * BINS] = negpad.astype(NPBF)

    wf = np.zeros((128, _CWF), np.float32)
    wf[:, _OFFF["B1S"]] = np.tile(f["B1"], 4)
    wf[:, _OFFF["B2S"]] = np.tile(f["B2"], 2)
    wf[:, _OFFF["B3S"]] = np.tile(f["B3"], 2)
    wf[:, _OFFF["B4S"]] = np.tile(f["B4"], 2)
    wf[:, _OFFF["B5S"]] = f["B5"]
    wf[:, _OFFF["B6S"]] = f["B6"]
    wf[:64, _OFFF["F1B"]] = f["F1B"]
    wf[:32, _OFFF["F2B"]] = f["F2B"]
    wf[:NCLS, _OFFF["F3B"]] = f["F3B"]
    wf[:, _OFFF["INV"]:_OFFF["INV"] + GROUPS * BINS] = inv[None, :]
    return wb, wf


def _pack_sel(f, nsup):
    """L1 selector: col block s = [64, 128]; out partition 32c+j gets
    W1[0, j] from x-row (4s+c)."""
    sel = np.zeros((64, nsup * 128), NPBF)
    w1 = f["W1"][0].astype(NPBF)
    for s in range(nsup):
        for c in range(4):
            r = (4 * s + c) % 64
            sel[r, s * 128 + 32 * c: s * 128 + 32 * c + 32] = w1
    return sel


# ------------------------------------------------------------- device build --
def _build(G2):
    NSUP = G2 // SUP
    NBLK = G2 // GRAN
    BPS = SUP // GRAN            # blocks per super (=128)
    assert BPS == 128 and G2 <= 64 * 512
    NCHUNK = NBLK // 128         # == NSUP

    nc = bacc.Bacc(None, target_bir_lowering=False)
    xs_d = nc.declare_dram_parameter("xs", [GROUPS, 64, 512], BF16,
                                     isOutput=False)
    a_d = nc.declare_dram_parameter("amat", [GROUPS, 128, NBLK], BF16,
                                    isOutput=False)
    sel_d = nc.declare_dram_parameter("selc", [64, NSUP * 128], BF16,
                                      isOutput=False)
    wb_d = nc.declare_dram_parameter("wbf", [128, _CWB], BF16, isOutput=False)
    wf_d = nc.declare_dram_parameter("wfp", [128, _CWF], F32, isOutput=False)
    out_d = nc.declare_dram_parameter("out", [NCLS, GROUPS * BINS], F32,
                                      isOutput=True)

    with ExitStack() as ctx:
        tc = ctx.enter_context(tile.TileContext(nc))
        cpool = ctx.enter_context(tc.tile_pool(name="const", bufs=1))
        xpool = ctx.enter_context(tc.tile_pool(name="xg", bufs=2))
        zpool = ctx.enter_context(tc.tile_pool(name="zq", bufs=2))
        gpool = ctx.enter_context(tc.tile_pool(name="gacc", bufs=1))

        wbsb = cpool.tile([128, _CWB], BF16)
        nc.sync.dma_start(wbsb[:], wb_d[:])
        wfsb = cpool.tile([128, _CWF], F32)
        nc.sync.dma_start(wfsb[:], wf_d[:])
        selsb = cpool.tile([64, NSUP * 128], BF16)
        nc.sync.dma_start(selsb[:], sel_d[:])

        def WB(name, k, m):
            o = _OFFB[name]
            return wbsb[0:k, o:o + m]

        def WF(name, k, m=1):
            o = _OFFF[name]
            return wfsb[0:k, o:o + m]

        bd2a, bd2b = WB("BD2A", 128, 128), WB("BD2B", 128, 128)
        bd3, bd4 = WB("BD3", 128, 128), WB("BD4", 128, 128)
        w5r, w6 = WB("W5R", 128, 128), WB("W6", 128, 128)
        f1, f2, f3 = WB("F1", 128, 64), WB("F2", 64, 32), WB("F3", 32, NCLS)
        c6row = WB("C6", 1, 128)
        negrow = WB("NEG", 1, GROUPS * BINS)
        b1s, b2s, b3s = WF("B1S", 128), WF("B2S", 128), WF("B3S", 128)
        b4s, b5s, b6s = WF("B4S", 128), WF("B5S", 128), WF("B6S", 128)
        f1b, f2b, f3b = WF("F1B", 64), WF("F2B", 32), WF("F3B", NCLS)
        invsb = WF("INV", 128, GROUPS * BINS)

        bs_t, bt_t, a_t = [], [], []
        for g in range(GROUPS):
            bs_t.append(gpool.tile([128, NBLK], BF16, name=f"bs{g}"))
            bt_t.append(gpool.tile([128, NBLK], BF16, name=f"bt{g}"))
            a_t.append(gpool.tile([128, NBLK], BF16, name=f"amat{g}"))
        gsb = gpool.tile([128, GROUPS * BINS], BF16, name="gsb")

        for g in range(GROUPS):
            nc.sync.dma_start(a_t[g][:], a_d[g])

        supers = [(g, s) for g in range(GROUPS) for s in range(NSUP)]
        K = len(supers)
        st = {}          # k -> dict of live tiles
        xgs = {}

        def load_x(g):
            xg = xpool.tile([64, 512], BF16, tag="xg", name=f"xg{g}")
            nc.sync.dma_start(xg[:], xs_d[g])
            xgs[g] = xg

        def stage_A(k):
            g, s = supers[k]
            d = st.setdefault(k, {})
            p1 = psS.tile([128, 1024], F32, tag="ps", name=f"p1_{k}")
            nc.tensor.matmul(p1[:, 0:512], selsb[:, s * 128:(s + 1) * 128],
                             xgs[g][:], start=True, stop=True)
            z1q = zpool.tile([128, 512], BF16, tag="z1", name=f"z1_{k}")
            nc.scalar.activation(z1q[:], p1[:, 0:512], RELU, bias=b1s)
            d["z1"] = z1q

        def stage_L2(k):
            d = st[k]
            p2 = psS.tile([128, 1024], F32, tag="ps", name=f"p2_{k}")
            nc.tensor.matmul(p2[:, 0:512], bd2a, d["z1"][:],
                             start=True, stop=True)
            nc.tensor.matmul(p2[:, 512:1024], bd2b, d["z1"][:],
                             start=True, stop=True)
            z2q = zpool.tile([128, 1024], BF16, tag="z2", name=f"z2_{k}",
                             bufs=3)
            nc.vector.tensor_scalar(z2q[:], p2[:], b2s, 0.0, ALU.add, ALU.max)
            d["z2"] = z2q

        def stage_L3(k):
            d = st[k]
            p3 = psS.tile([128, 1024], F32, tag="ps", name=f"p3_{k}")
            nc.tensor.matmul(p3[:, 0:512], bd3, d["z2"][:, 0:512],
                             start=True, stop=True)
            nc.tensor.matmul(p3[:, 512:1024], bd3, d["z2"][:, 512:1024],
                             start=True, stop=True)
            z3q = zpool.tile([128, 1024], BF16, tag="z3", name=f"z3_{k}",
                             bufs=3)
            nc.vector.tensor_scalar(z3q[:], p3[:], b3s, 0.0, ALU.add, ALU.max)
            d["z3"] = z3q

        def stage_L4(k):
            d = st[k]
            p4 = psS.tile([128, 1024], F32, tag="ps", name=f"p4_{k}")
            nc.tensor.matmul(p4[:, 0:512], bd4, d["z3"][:, 0:512],
                             start=True, stop=True)
            nc.tensor.matmul(p4[:, 512:1024], bd4, d["z3"][:, 512:1024],
                             start=True, stop=True)
            z4q = zpool.tile([128, 1024], BF16, tag="z4", name=f"z4_{k}",
                             bufs=3)
            nc.vector.tensor_scalar(z4q[:], p4[:], b4s, 0.0, ALU.add, ALU.max)
            d["z4"] = z4q

        def stage_L5(k):
            d = st[k]
            p5 = psB.tile([128, 2048], F32, tag="big", name=f"p5_{k}")
            z4q = d["z4"]
            for ch in range(4):
                lo, hi = (0, 64) if ch % 2 == 0 else (64, 128)
                col = (ch // 2) * 512
                nc.tensor.matmul(p5[:, ch * 512:(ch + 1) * 512],
                                 w5r[lo:hi, :], z4q[lo:hi, col:col + 512],
                                 start=True, stop=True)
            z5q = zpool.tile([128, 2048], BF16, tag="z5", name=f"z5_{k}",
                             bufs=3)
            nc.scalar.activation(z5q[:], p5[:], RELU, bias=b5s)
            d["z5"] = z5q

        def stage_L6(k):
            d = st[k]
            p6 = psB.tile([128, 2048], F32, tag="big", name=f"p6_{k}")
            z5q = d["z5"]
            for ch in range(4):
                nc.tensor.matmul(p6[:, ch * 512:(ch + 1) * 512],
                                 w6, z5q[:, ch * 512:(ch + 1) * 512],
                                 start=True, stop=True)
            z6q = zpool.tile([128, 2048], BF16, tag="z6", name=f"z6_{k}",
                             bufs=3)
            nc.scalar.activation(z6q[:], p6[:], RELU, bias=b6s)
            d["z6"] = z6q

        def stage_R(k):
            """Fold-tree block sums: gpsimd does the two big folds, DVE the
            two small ones.  Blocks are strided (node m of block j sits at
            column j + 128*m of the super), so halving folds preserve them."""
            g, s = supers[k]
            z6q = st[k]["z6"]
            t1 = zpool.tile([128, 1024], FP16, tag="t1", name=f"t1_{k}")
            nc.gpsimd.tensor_tensor(t1[:], z6q[:, 0:1024], z6q[:, 1024:2048],
                                    ALU.add)
            t2 = zpool.tile([128, 512], FP16, tag="t2", name=f"t2_{k}")
            nc.gpsimd.tensor_tensor(t2[:], t1[:, 0:512], t1[:, 512:1024],
                                    ALU.add)
            t3 = zpool.tile([128, 256], FP16, tag="t3", name=f"t3_{k}")
            nc.gpsimd.tensor_tensor(t3[:], t2[:, 0:256], t2[:, 256:512],
                                    ALU.add)
            with nc.allow_low_precision("bf16 block sums; pooled means "
                                        "tolerate 0.4% noise"):
                nc.vector.tensor_tensor(bs_t[g][:, s * 128:(s + 1) * 128],
                                        t3[:, 0:128], t3[:, 128:256], ALU.add)
            nc.sync.dma_start_transpose(
                bt_t[g][:, s * 128:(s + 1) * 128],
                bs_t[g][:, s * 128:(s + 1) * 128])
            del st[k]

        def agg_group(g, pool, tag):
            sgt = pool.tile([128, 2048], F32, tag=tag, name=f"sg{g}") \
                if tag == "big" else pool.tile([128, BINS], F32, tag=tag,
                                               name=f"sg{g}")
            sg = sgt[:, 0:BINS]
            for c in range(NCHUNK):
                nc.tensor.matmul(sg, bt_t[g][:, c * 128:(c + 1) * 128],
                                 a_t[g][:, c * 128:(c + 1) * 128],
                                 start=(c == 0), stop=False,
                                 skip_group_check=True)
            nc.tensor.matmul(sg, c6row, negrow[:, g * BINS:(g + 1) * BINS],
                             start=False, stop=True, skip_group_check=True)
            nc.vector.tensor_tensor(
                gsb[:, g * BINS:(g + 1) * BINS], sg,
                invsb[:, g * BINS:(g + 1) * BINS], ALU.mult)

        with tc.tile_pool(name="psS", bufs=2, space="PSUM") as psS, \
             tc.tile_pool(name="psB", bufs=1, space="PSUM") as psB:
            load_x(0)
            for k in range(K + 6):
                if k < K:
                    if k + 4 < K and supers[k + 4][1] == 0:
                        load_x(supers[k + 4][0])
                    stage_A(k)
                if 0 <= k - 1 < K:
                    stage_L3(k - 1)
                if k < K:
                    stage_L2(k)
                if 0 <= k - 2 < K:
                    stage_L4(k - 2)
                if 0 <= k - 3 < K:
                    stage_L5(k - 3)
                if 0 <= k - 4 < K:
                    stage_L6(k - 4)
                if 0 <= k - 5 < K:
                    stage_R(k - 5)

        # ---------------- end phase: aggregation + graph MLP ----------------
        with tc.tile_pool(name="psA", bufs=2, space="PSUM") as psA:
            for g in range(GROUPS):
                agg_group(g, psA, "agg")

            pf1 = psA.tile([64, 512], F32, tag="agg", name="pf1")
            nc.tensor.matmul(pf1[:], f1, gsb[:], start=True, stop=True)
            a1 = zpool.tile([64, 512], BF16, tag="a1")
            nc.scalar.activation(a1[:], pf1[:], RELU, bias=f1b)
            pf2 = psA.tile([32, 512], F32, tag="agg", name="pf2")
            nc.tensor.matmul(pf2[:], f2, a1[:], start=True, stop=True)
            a2 = zpool.tile([32, 512], BF16, tag="a2")
            nc.scalar.activation(a2[:], pf2[:], RELU, bias=f2b)
            pf3 = psA.tile([NCLS, 512], F32, tag="agg", name="pf3")
            nc.tensor.matmul(pf3[:], f3, a2[:], start=True, stop=True)
            osb = zpool.tile([NCLS, 512], F32, tag="osb")
            nc.vector.tensor_scalar(osb[:], pf3[:], f3b, None, ALU.add)
            nc.sync.dma_start(out_d[:], osb[:])

    nc.compile()
    return nc


# -------------------------------------------------------------------- entry --
def kernel(**inputs):
    global LAST_RESULT
    x = np.asarray(inputs["x"], np.float32)
    batch = np.asarray(inputs["batch"], np.int64)
    B = int(np.asarray(inputs["num_graphs"]))
    assert B == NCORES * GROUPS * BINS, f"unexpected num_graphs {B}"
    T = x.shape[0]

    params = {k: np.asarray(v, np.float32) for k, v in inputs.items()
              if k not in ("x", "batch", "num_graphs")}
    f = _fold(params)
    c6 = _c6(f)

    counts = np.bincount(batch, minlength=B).astype(np.int64)
    nblk = -(-counts // GRAN)
    pad = (nblk * GRAN - counts).astype(np.float32)
    NCG = NCORES * GROUPS
    nblk_cg = nblk.reshape(NCG, BINS)
    blkstart = np.zeros((NCG, BINS), np.int64)
    blkstart[:, 1:] = np.cumsum(nblk_cg, axis=1)[:, :-1]
    P_cg = nblk_cg.sum(axis=1) * GRAN
    G2 = int(-(-int(P_cg.max()) // SUP) * SUP)
    NBLK = G2 // GRAN

    # padded positions
    bounds = np.zeros(B + 1, np.int64)
    bounds[1:] = np.cumsum(counts)
    within = np.arange(T, dtype=np.int64) - bounds[batch]
    cg_of = batch // BINS
    ppos = blkstart[cg_of, batch % BINS] * GRAN + within
    # strided in-super layout: node m of block j -> column j + 128*m, so the
    # device fold-tree (halving adds) preserves block identity
    q = ppos % SUP
    dpos = (ppos // SUP) * SUP + (q % GRAN) * (SUP // GRAN) + q // GRAN
    xp = np.zeros((NCG, 64 * 512), np.float32)
    xp[cg_of, dpos] = x
    xs = xp.reshape(NCORES, GROUPS, 64, 512).astype(NPBF)

    # block -> bin interval matrix, chunk-transposed device layout
    amat = np.zeros((NCG, NBLK, BINS), NPBF)
    for cg in range(NCG):
        owner = np.repeat(np.arange(BINS), nblk_cg[cg])
        amat[cg, np.arange(owner.size), owner] = NPBF(1.0)
    amat = amat.reshape(NCORES, GROUPS, NBLK // 128, 128, BINS)
    amat = np.ascontiguousarray(amat.transpose(0, 1, 3, 2, 4)).reshape(
        NCORES, GROUPS, 128, NBLK)

    negpad = (-pad).reshape(NCORES, GROUPS * BINS)
    inv = (1.0 / np.maximum(counts, 1)).astype(np.float32).reshape(
        NCORES, GROUPS * BINS)

    sel = _pack_sel(f, G2 // SUP)

    if G2 not in _NC_CACHE:
        _NC_CACHE[G2] = _build(G2)
    nc = _NC_CACHE[G2]

    in_maps = []
    for c in range(NCORES):
        wb, wf = _pack_consts(f, c6, negpad[c], inv[c])
        in_maps.append({"xs": xs[c], "amat": amat[c], "selc": sel,
                       "wbf": wb, "wfp": wf})
    res = run_bass_kernel_spmd(nc, in_maps, core_ids=list(range(NCORES)))
    LAST_RESULT = res
    outs = np.stack([res.results[i]["out"] for i in range(NCORES)])
    return np.ascontiguousarray(
        outs.transpose(0, 2, 1).reshape(B, NCLS)).astype(np.float32)


# revision 16
# speedup vs baseline: 1.4689x; 1.3612x over previous
"""Trainium2 Bass kernel for AdaptedEnzymeModel (per-node MLP -> segment mean
pool -> graph MLP), SPMD over 8 NeuronCores.  v2.1: blockdiag layers, padded
16-node block pooling, software-pipelined supers.

Design
------
* BN (eval) affines folded into adjacent Linears on host; device runs bf16
  Linear+ReLU chains with fp32 PSUM accumulate.
* Nodes sharded at graph boundaries: core c owns graphs [512c, 512c+512),
  split into 4 groups of 128 graphs.  Each graph is padded to a multiple of
  16 nodes (pad nodes x=0); each group padded to G2 (multiple of 2048).
* Per 2048-node "super": L1 is ONE 64x128x512 matmul (4 channels x 32 feats
  packed in partitions; selector stationary picks the 4 x-rows).  L2-L4 run
  as full-array matmuls with block-diagonal stationaries diag(W,W) on the
  dual-channel [128,512] layout.  L5/L6 are weight-stationary, 4 matmuls each
  into one 4-bank [128,2048] PSUM tile, evacuated by a single ACT op
  (relu + per-partition bias).
* Pooling: one DVE tensor_reduce per super sums each 16-col block of z6
  ([128,128,16] -> [128,128] bf16) into the group block-sum buffer; that
  128-col chunk is immediately 128x128-transposed by the DMA xbar.  Pad-node
  garbage (x=0 -> constant c6 vector) is removed exactly by a rank-1
  c6 (x) (-padcount) correction matmul.  End phase: per group 16 accumulating
  matmuls against the host-built block->graph interval matrix -> fp32 means
  -> graph MLP -> [7, 512] out.
* Emission is software-pipelined: iteration k issues L1-L4 of super k,
  L5/L6 of super k-1 and the reduce+transpose of super k-2, so ACT/DVE
  queues always hold ready work.
"""

import numpy as np
import ml_dtypes
from contextlib import ExitStack

import concourse.bass as bass
import concourse.tile as tile
from concourse import bacc, mybir
from concourse.bass_utils import run_bass_kernel_spmd

NCORES = 8
GROUPS = 4
BINS = 128
GRAN = 16
SUP = 2048
NCLS = 7
EPS = 1e-5
F32 = mybir.dt.float32
BF16 = mybir.dt.bfloat16
FP16 = mybir.dt.float16
NPBF = ml_dtypes.bfloat16
RELU = mybir.ActivationFunctionType.Relu
ALU = mybir.AluOpType
AXX = mybir.AxisListType.X

LAST_RESULT = None
_NC_CACHE = {}


def _ensure_ntff_hook():
    """bass_utils' trace path needs antenv.axon_hooks, which this image's
    antenv package lacks.  Register a shim backed by trn_agent_boot's ctypes
    NTFF driver so BASS_TRACE=1 yields exec_time_ns.  Degrades silently."""
    import sys
    import types
    try:
        import antenv
        if "antenv.axon_hooks" in sys.modules:
            return
        mod = types.ModuleType("antenv.axon_hooks")
        mod._hook = None
        mod.set_axon_ntff_profile_hook = lambda h: setattr(mod, "_hook", h)
        mod.get_axon_ntff_profile_hook = lambda: mod._hook
        sys.modules["antenv.axon_hooks"] = mod
        antenv.axon_hooks = mod
        from trn_agent_boot.trn_boot import _ntff_profile_via_ctypes
        mod._hook = _ntff_profile_via_ctypes("/opt/axon/libaxon_pjrt.so")
    except Exception:
        pass


_ensure_ntff_hook()


# ---------------------------------------------------------------- host math --
def _fold(p):
    def aff(bn):
        g, b, m, v = bn[0], bn[1], bn[2], bn[3]
        s = g / np.sqrt(v + EPS)
        return s.astype(np.float32), (b - m * s).astype(np.float32)

    s1, t1 = aff(p["ne_bn1"]); s2, t2 = aff(p["ne_bn2"])
    sc1, tc1 = aff(p["cbn1"]); sc2, tc2 = aff(p["cbn2"])
    sf1, tf1 = aff(p["fbn1"]); sf2, tf2 = aff(p["fbn2"])
    f = {}
    f["W1"] = p["ne_w1"]; f["B1"] = p["ne_b1"]
    f["W2"] = s1[:, None] * p["ne_w2"]; f["B2"] = t1 @ p["ne_w2"] + p["ne_b2"]
    f["W3"] = s2[:, None] * p["c1a_w"]; f["B3"] = t2 @ p["c1a_w"] + p["c1a_b"]
    f["W4"] = p["c1b_w"];               f["B4"] = p["c1b_b"]
    f["W5"] = sc1[:, None] * p["c2a_w"]; f["B5"] = tc1 @ p["c2a_w"] + p["c2a_b"]
    f["W6"] = p["c2b_w"];               f["B6"] = p["c2b_b"]
    f["F1"] = sc2[:, None] * p["f1_w"]; f["F1B"] = tc2 @ p["f1_w"] + p["f1_b"]
    f["F2"] = sf1[:, None] * p["f2_w"]; f["F2B"] = tf1 @ p["f2_w"] + p["f2_b"]
    f["F3"] = sf2[:, None] * p["f3_w"]; f["F3B"] = tf2 @ p["f3_w"] + p["f3_b"]
    return {k: np.asarray(v, np.float32) for k, v in f.items()}


def _c6(f):
    """Feature vector a pad node (x=0) produces at z6, replicating device
    rounding (bf16 weights/activations, fp32 accumulate)."""
    relu = lambda a: np.maximum(a, 0.0)
    z = relu(f["B1"]).astype(NPBF)
    for w, b in (("W2", "B2"), ("W3", "B3"), ("W4", "B4"), ("W5", "B5"),
                 ("W6", "B6")):
        z = z.astype(np.float32) @ f[w].astype(NPBF).astype(np.float32)
        z = relu(z + f[b]).astype(NPBF)
    return z.astype(np.float32)


# bf16 const block layout
def _layout_bf():
    off, c = {}, 0
    for name, ncols in [("BD2A", 128), ("BD2B", 128), ("BD3", 128),
                        ("BD4", 128), ("W5R", 128), ("W6", 128),
                        ("F1", 64), ("F2", 32), ("F3", NCLS),
                        ("C6", 128), ("NEG", GROUPS * BINS)]:
        off[name] = c
        c += ncols
    return off, c


def _layout_fp():
    off, c = {}, 0
    for name, ncols in [("B1S", 1), ("B2S", 1), ("B3S", 1), ("B4S", 1),
                        ("B5S", 1), ("B6S", 1), ("F1B", 1), ("F2B", 1),
                        ("F3B", 1), ("INV", GROUPS * BINS)]:
        off[name] = c
        c += ncols
    return off, c


_OFFB, _CWB = _layout_bf()
_OFFF, _CWF = _layout_fp()


def _pack_consts(f, c6, negpad, inv):
    """negpad [512] f32, inv [512] f32 per core."""
    wb = np.zeros((128, _CWB), NPBF)

    def putb(name, arr):
        wb[:arr.shape[0], _OFFB[name]:_OFFB[name] + arr.shape[1]] = \
            arr.astype(NPBF)

    bd2a = np.zeros((128, 128), np.float32)
    bd2a[0:32, 0:64] = f["W2"]
    bd2a[32:64, 64:128] = f["W2"]
    bd2b = np.zeros((128, 128), np.float32)
    bd2b[64:96, 0:64] = f["W2"]
    bd2b[96:128, 64:128] = f["W2"]
    putb("BD2A", bd2a)
    putb("BD2B", bd2b)
    for nm, w in (("BD3", "W3"), ("BD4", "W4")):
        bd = np.zeros((128, 128), np.float32)
        bd[0:64, 0:64] = f[w]
        bd[64:128, 64:128] = f[w]
        putb(nm, bd)
    putb("W5R", np.tile(f["W5"], (2, 1)))
    putb("W6", f["W6"])
    putb("F1", f["F1"])
    putb("F2", f["F2"])
    putb("F3", f["F3"])
    wb[0, _OFFB["C6"]:_OFFB["C6"] + 128] = c6.astype(NPBF)
    wb[0, _OFFB["NEG"]:_OFFB["NEG"] + GROUPS * BINS] = negpad.astype(NPBF)

    wf = np.zeros((128, _CWF), np.float32)
    wf[:, _OFFF["B1S"]] = np.tile(f["B1"], 4)
    wf[:, _OFFF["B2S"]] = np.tile(f["B2"], 2)
    wf[:, _OFFF["B3S"]] = np.tile(f["B3"], 2)
    wf[:, _OFFF["B4S"]] = np.tile(f["B4"], 2)
    wf[:, _OFFF["B5S"]] = f["B5"]
    wf[:, _OFFF["B6S"]] = f["B6"]
    wf[:64, _OFFF["F1B"]] = f["F1B"]
    wf[:32, _OFFF["F2B"]] = f["F2B"]
    wf[:NCLS, _OFFF["F3B"]] = f["F3B"]
    wf[:, _OFFF["INV"]:_OFFF["INV"] + GROUPS * BINS] = inv[None, :]
    return wb, wf


def _pack_sel(f, nsup):
    """L1 selector: col block s = [64, 128]; out partition 32c+j gets
    W1[0, j] from x-row (4s+c)."""
    sel = np.zeros((64, nsup * 128), NPBF)
    w1 = f["W1"][0].astype(NPBF)
    for s in range(nsup):
        for c in range(4):
            r = (4 * s + c) % 64
            sel[r, s * 128 + 32 * c: s * 128 + 32 * c + 32] = w1
    return sel


# ------------------------------------------------------------- device build --
def _build(G2):
    NSUP = G2 // SUP
    NBLK = G2 // GRAN
    BPS = SUP // GRAN            # blocks per super (=128)
    assert BPS == 128 and G2 <= 64 * 512
    NCHUNK = NBLK // 128         # == NSUP

    nc = bacc.Bacc(None, target_bir_lowering=False)
    xs_d = nc.declare_dram_parameter("xs", [GROUPS, 64, 512], BF16,
                                     isOutput=False)
    a_d = nc.declare_dram_parameter("amat", [GROUPS, 128, NBLK], BF16,
                                    isOutput=False)
    sel_d = nc.declare_dram_parameter("selc", [64, NSUP * 128], BF16,
                                      isOutput=False)
    wb_d = nc.declare_dram_parameter("wbf", [128, _CWB], BF16, isOutput=False)
    wf_d = nc.declare_dram_parameter("wfp", [128, _CWF], F32, isOutput=False)
    out_d = nc.declare_dram_parameter("out", [NCLS, GROUPS * BINS], F32,
                                      isOutput=True)

    with ExitStack() as ctx:
        tc = ctx.enter_context(tile.TileContext(nc))
        cpool = ctx.enter_context(tc.tile_pool(name="const", bufs=1))
        xpool = ctx.enter_context(tc.tile_pool(name="xg", bufs=2))
        zpool = ctx.enter_context(tc.tile_pool(name="zq", bufs=2))
        gpool = ctx.enter_context(tc.tile_pool(name="gacc", bufs=1))

        wbsb = cpool.tile([128, _CWB], BF16)
        nc.sync.dma_start(wbsb[:], wb_d[:])
        wfsb = cpool.tile([128, _CWF], F32)
        nc.sync.dma_start(wfsb[:], wf_d[:])
        selsb = cpool.tile([64, NSUP * 128], BF16)
        nc.sync.dma_start(selsb[:], sel_d[:])

        def WB(name, k, m):
            o = _OFFB[name]
            return wbsb[0:k, o:o + m]

        def WF(name, k, m=1):
            o = _OFFF[name]
            return wfsb[0:k, o:o + m]

        bd2a, bd2b = WB("BD2A", 128, 128), WB("BD2B", 128, 128)
        bd3, bd4 = WB("BD3", 128, 128), WB("BD4", 128, 128)
        w5r, w6 = WB("W5R", 128, 128), WB("W6", 128, 128)
        f1, f2, f3 = WB("F1", 128, 64), WB("F2", 64, 32), WB("F3", 32, NCLS)
        c6row = WB("C6", 1, 128)
        negrow = WB("NEG", 1, GROUPS * BINS)
        b1s, b2s, b3s = WF("B1S", 128), WF("B2S", 128), WF("B3S", 128)
        b4s, b5s, b6s = WF("B4S", 128), WF("B5S", 128), WF("B6S", 128)
        f1b, f2b, f3b = WF("F1B", 64), WF("F2B", 32), WF("F3B", NCLS)
        invsb = WF("INV", 128, GROUPS * BINS)

        bs_t, bt_t, a_t = [], [], []
        for g in range(GROUPS):
            bs_t.append(gpool.tile([128, NBLK], BF16, name=f"bs{g}"))
            bt_t.append(gpool.tile([128, NBLK], BF16, name=f"bt{g}"))
            a_t.append(gpool.tile([128, NBLK], BF16, name=f"amat{g}"))
        gsb = gpool.tile([128, GROUPS * BINS], BF16, name="gsb")

        for g in range(GROUPS):
            nc.sync.dma_start(a_t[g][:], a_d[g])

        supers = [(g, s) for g in range(GROUPS) for s in range(NSUP)]
        K = len(supers)
        st = {}          # k -> dict of live tiles
        xgs = {}

        def load_x(g):
            xg = xpool.tile([64, 512], BF16, tag="xg", name=f"xg{g}")
            nc.sync.dma_start(xg[:], xs_d[g])
            xgs[g] = xg

        def stage_A(k):
            g, s = supers[k]
            d = st.setdefault(k, {})
            p1 = psS.tile([128, 1024], F32, tag="ps", name=f"p1_{k}")
            nc.tensor.matmul(p1[:, 0:512], selsb[:, s * 128:(s + 1) * 128],
                             xgs[g][:], start=True, stop=True)
            z1q = zpool.tile([128, 512], BF16, tag="z1", name=f"z1_{k}")
            nc.vector.tensor_scalar(z1q[:], p1[:, 0:512], b1s, 0.0,
                                    ALU.add, ALU.max)
            d["z1"] = z1q

        def stage_L2(k):
            d = st[k]
            p2 = psS.tile([128, 1024], F32, tag="ps", name=f"p2_{k}")
            nc.tensor.matmul(p2[:, 0:512], bd2a, d["z1"][:],
                             start=True, stop=True)
            nc.tensor.matmul(p2[:, 512:1024], bd2b, d["z1"][:],
                             start=True, stop=True)
            z2q = zpool.tile([128, 1024], BF16, tag="z2", name=f"z2_{k}",
                             bufs=3)
            nc.vector.tensor_scalar(z2q[:], p2[:], b2s, 0.0, ALU.add, ALU.max)
            d["z2"] = z2q

        def stage_L3(k):
            d = st[k]
            p3 = psS.tile([128, 1024], F32, tag="ps", name=f"p3_{k}")
            nc.tensor.matmul(p3[:, 0:512], bd3, d["z2"][:, 0:512],
                             start=True, stop=True)
            nc.tensor.matmul(p3[:, 512:1024], bd3, d["z2"][:, 512:1024],
                             start=True, stop=True)
            z3q = zpool.tile([128, 1024], BF16, tag="z3", name=f"z3_{k}",
                             bufs=3)
            nc.vector.tensor_scalar(z3q[:], p3[:], b3s, 0.0, ALU.add, ALU.max)
            d["z3"] = z3q

        def stage_L4(k):
            d = st[k]
            p4 = psS.tile([128, 1024], F32, tag="ps", name=f"p4_{k}")
            nc.tensor.matmul(p4[:, 0:512], bd4, d["z3"][:, 0:512],
                             start=True, stop=True)
            nc.tensor.matmul(p4[:, 512:1024], bd4, d["z3"][:, 512:1024],
                             start=True, stop=True)
            z4q = zpool.tile([128, 1024], BF16, tag="z4", name=f"z4_{k}",
                             bufs=3)
            nc.vector.tensor_scalar(z4q[:], p4[:], b4s, 0.0, ALU.add, ALU.max)
            d["z4"] = z4q

        def stage_L5(k):
            """Two half-super [128,1024] PSUM tiles so the psB slots rotate
            at half-super granularity: ACT evacuates one half while the PE
            streams the other."""
            d = st[k]
            z4q = d["z4"]
            z5h = []
            for h in range(2):
                p5 = psB.tile([128, 1024], F32, tag="big", name=f"p5{h}_{k}")
                for ch in (2 * h, 2 * h + 1):
                    lo, hi = (0, 64) if ch % 2 == 0 else (64, 128)
                    col = (ch // 2) * 512
                    nc.tensor.matmul(p5[:, (ch % 2) * 512:(ch % 2) * 512 + 512],
                                     w5r[lo:hi, :], z4q[lo:hi, col:col + 512],
                                     start=True, stop=True)
                zq = zpool.tile([128, 1024], BF16, tag=f"z5{h}",
                                name=f"z5{h}_{k}", bufs=3)
                nc.scalar.activation(zq[:], p5[:], RELU, bias=b5s)
                z5h.append(zq)
            d["z5"] = z5h

        def stage_L6(k):
            d = st[k]
            z5h = d["z5"]
            z6h = []
            for h in range(2):
                p6 = psB.tile([128, 1024], F32, tag="big", name=f"p6{h}_{k}")
                for c in range(2):
                    nc.tensor.matmul(p6[:, c * 512:(c + 1) * 512],
                                     w6, z5h[h][:, c * 512:(c + 1) * 512],
                                     start=True, stop=True)
                zq = zpool.tile([128, 1024], BF16, tag=f"z6{h}",
                                name=f"z6{h}_{k}", bufs=3)
                nc.scalar.activation(zq[:], p6[:], RELU, bias=b6s)
                z6h.append(zq)
            d["z6"] = z6h

        def stage_R(k):
            """Fold-tree block sums: gpsimd does the two big folds, DVE the
            two small ones.  Blocks are strided (node m of block j sits at
            column j + 128*m of the super), so halving folds preserve them."""
            g, s = supers[k]
            z6h = st[k]["z6"]
            t1 = zpool.tile([128, 1024], FP16, tag="t1", name=f"t1_{k}")
            nc.gpsimd.tensor_tensor(t1[:], z6h[0][:], z6h[1][:], ALU.add)
            t2 = zpool.tile([128, 512], FP16, tag="t2", name=f"t2_{k}")
            nc.gpsimd.tensor_tensor(t2[:], t1[:, 0:512], t1[:, 512:1024],
                                    ALU.add)
            t3 = zpool.tile([128, 256], FP16, tag="t3", name=f"t3_{k}")
            nc.gpsimd.tensor_tensor(t3[:], t2[:, 0:256], t2[:, 256:512],
                                    ALU.add)
            with nc.allow_low_precision("bf16 block sums; pooled means "
                                        "tolerate 0.4% noise"):
                nc.vector.tensor_tensor(bs_t[g][:, s * 128:(s + 1) * 128],
                                        t3[:, 0:128], t3[:, 128:256], ALU.add)
            nc.sync.dma_start_transpose(
                bt_t[g][:, s * 128:(s + 1) * 128],
                bs_t[g][:, s * 128:(s + 1) * 128])
            del st[k]

        def agg_group(g, pool, tag):
            sgt = pool.tile([128, 2048], F32, tag=tag, name=f"sg{g}") \
                if tag == "big" else pool.tile([128, BINS], F32, tag=tag,
                                               name=f"sg{g}")
            sg = sgt[:, 0:BINS]
            for c in range(NCHUNK):
                nc.tensor.matmul(sg, bt_t[g][:, c * 128:(c + 1) * 128],
                                 a_t[g][:, c * 128:(c + 1) * 128],
                                 start=(c == 0), stop=False,
                                 skip_group_check=True)
            nc.tensor.matmul(sg, c6row, negrow[:, g * BINS:(g + 1) * BINS],
                             start=False, stop=True, skip_group_check=True)
            nc.vector.tensor_tensor(
                gsb[:, g * BINS:(g + 1) * BINS], sg,
                invsb[:, g * BINS:(g + 1) * BINS], ALU.mult)

        with tc.tile_pool(name="psS", bufs=2, space="PSUM") as psS, \
             tc.tile_pool(name="psB", bufs=2, space="PSUM") as psB:
            load_x(0)
            for k in range(K + 6):
                if k < K:
                    if k + 4 < K and supers[k + 4][1] == 0:
                        load_x(supers[k + 4][0])
                    stage_A(k)
                if 0 <= k - 1 < K:
                    stage_L3(k - 1)
                if k < K:
                    stage_L2(k)
                if 0 <= k - 2 < K:
                    stage_L4(k - 2)
                if 0 <= k - 3 < K:
                    stage_L5(k - 3)
                if 0 <= k - 4 < K:
                    stage_L6(k - 4)
                if 0 <= k - 5 < K:
                    stage_R(k - 5)

        # ---------------- end phase: aggregation + graph MLP ----------------
        with tc.tile_pool(name="psA", bufs=2, space="PSUM") as psA:
            for g in range(GROUPS):
                agg_group(g, psA, "agg")

            pf1 = psA.tile([64, 512], F32, tag="agg", name="pf1")
            nc.tensor.matmul(pf1[:], f1, gsb[:], start=True, stop=True)
            a1 = zpool.tile([64, 512], BF16, tag="a1")
            nc.scalar.activation(a1[:], pf1[:], RELU, bias=f1b)
            pf2 = psA.tile([32, 512], F32, tag="agg", name="pf2")
            nc.tensor.matmul(pf2[:], f2, a1[:], start=True, stop=True)
            a2 = zpool.tile([32, 512], BF16, tag="a2")
            nc.scalar.activation(a2[:], pf2[:], RELU, bias=f2b)
            pf3 = psA.tile([NCLS, 512], F32, tag="agg", name="pf3")
            nc.tensor.matmul(pf3[:], f3, a2[:], start=True, stop=True)
            osb = zpool.tile([NCLS, 512], F32, tag="osb")
            nc.vector.tensor_scalar(osb[:], pf3[:], f3b, None, ALU.add)
            nc.sync.dma_start(out_d[:], osb[:])

    nc.compile()
    return nc


# -------------------------------------------------------------------- entry --
def kernel(**inputs):
    global LAST_RESULT
    x = np.asarray(inputs["x"], np.float32)
    batch = np.asarray(inputs["batch"], np.int64)
    B = int(np.asarray(inputs["num_graphs"]))
    assert B == NCORES * GROUPS * BINS, f"unexpected num_graphs {B}"
    T = x.shape[0]

    params = {k: np.asarray(v, np.float32) for k, v in inputs.items()
              if k not in ("x", "batch", "num_graphs")}
    f = _fold(params)
    c6 = _c6(f)

    counts = np.bincount(batch, minlength=B).astype(np.int64)
    nblk = -(-counts // GRAN)
    pad = (nblk * GRAN - counts).astype(np.float32)
    NCG = NCORES * GROUPS
    nblk_cg = nblk.reshape(NCG, BINS)
    blkstart = np.zeros((NCG, BINS), np.int64)
    blkstart[:, 1:] = np.cumsum(nblk_cg, axis=1)[:, :-1]
    P_cg = nblk_cg.sum(axis=1) * GRAN
    G2 = int(-(-int(P_cg.max()) // SUP) * SUP)
    NBLK = G2 // GRAN

    # padded positions
    bounds = np.zeros(B + 1, np.int64)
    bounds[1:] = np.cumsum(counts)
    within = np.arange(T, dtype=np.int64) - bounds[batch]
    cg_of = batch // BINS
    ppos = blkstart[cg_of, batch % BINS] * GRAN + within
    # strided in-super layout: node m of block j -> column j + 128*m, so the
    # device fold-tree (halving adds) preserves block identity
    q = ppos % SUP
    dpos = (ppos // SUP) * SUP + (q % GRAN) * (SUP // GRAN) + q // GRAN
    xp = np.zeros((NCG, 64 * 512), np.float32)
    xp[cg_of, dpos] = x
    xs = xp.reshape(NCORES, GROUPS, 64, 512).astype(NPBF)

    # block -> bin interval matrix, chunk-transposed device layout
    amat = np.zeros((NCG, NBLK, BINS), NPBF)
    for cg in range(NCG):
        owner = np.repeat(np.arange(BINS), nblk_cg[cg])
        amat[cg, np.arange(owner.size), owner] = NPBF(1.0)
    amat = amat.reshape(NCORES, GROUPS, NBLK // 128, 128, BINS)
    amat = np.ascontiguousarray(amat.transpose(0, 1, 3, 2, 4)).reshape(
        NCORES, GROUPS, 128, NBLK)

    negpad = (-pad).reshape(NCORES, GROUPS * BINS)
    inv = (1.0 / np.maximum(counts, 1)).astype(np.float32).reshape(
        NCORES, GROUPS * BINS)

    sel = _pack_sel(f, G2 // SUP)

    if G2 not in _NC_CACHE:
        _NC_CACHE[G2] = _build(G2)
    nc = _NC_CACHE[G2]

    in_maps = []
    for c in range(NCORES):
        wb, wf = _pack_consts(f, c6, negpad[c], inv[c])
        in_maps.append({"xs": xs[c], "amat": amat[c], "selc": sel,
                       "wbf": wb, "wfp": wf})
    res = run_bass_kernel_spmd(nc, in_maps, core_ids=list(range(NCORES)))
    LAST_RESULT = res
    outs = np.stack([res.results[i]["out"] for i in range(NCORES)])
    return np.ascontiguousarray(
        outs.transpose(0, 2, 1).reshape(B, NCLS)).astype(np.float32)
